# revision 1
# baseline (speedup 1.0000x reference)
"""Trainium2 Bass kernel for nn_Attention4D (B=64, DIM=384, 14x14, 8 heads).

Sharding: pure data-parallel over batch. 8 items per NeuronCore, 8 cores,
weights replicated; inputs sharded/gathered on host.

Per-item fused pipeline (all on one core):
  - Q,K,V projections as f32r matmuls (softmax scale folded into qw; Q/K/V
    moving operands are item-PAIRS, N=392/512, so f32r runs at full rate).
  - Attention maps live as 14 query-windows [(g,ns)=112, m=196], g = head
    index AFTER talking-heads-1. th1 is folded into 8 per-head-scaled copies
    of Q (VectorE tensor_scalar); the th1-mixed relative-position bias
    (host-gathered) is PRELOADED into PSUM via an identity-stationary matmul,
    and the scores accumulate on top.
  - exp on ScalarE directly PSUM->SBUF with accum_out = softmax denominators.
  - talking-heads-2 + softmax normalization + layout transpose in ONE normal
    matmul per (window, m-chunk): stationary = E-chunk, moving = T2R where
    T2R = (th2^T kron I_14) row-scaled by 1/denominators (built by one
    broadcast tensor_tensor per item). Output is directly [m, (h,ns)].
  - attn@v: stationary = V^T slices (V^T computed directly as x^T-stationary
    f32r matmuls, no transposes), moving = the scattered-evicted SM2T tiles.
    The depthwise-3x3 local branch (computed as 9 per-partition-scalar taps
    on VectorE in bf16) is preloaded into the same PSUM via an identity
    matmul. vb/th2_b corrections ride the per-partition eviction biases:
      o += vb*sum_g(th2[h,g]) + th2_b[h]*rowsum(v)  (rowsum from accum_out).
  - ReLU on eviction, final projection in bf16, + projb on final eviction.
"""

import sys

import numpy as np

sys.path.insert(0, "/opt/trn_rl_repo")

import ml_dtypes  # noqa: E402

import concourse.bacc as bacc  # noqa: E402
import concourse.mybir as mybir  # noqa: E402
import concourse.tile as tile  # noqa: E402
from concourse.bass_utils import run_bass_kernel_spmd  # noqa: E402

BF = mybir.dt.bfloat16
F32 = mybir.dt.float32
F32R = mybir.dt.float32r
F8 = mybir.dt.float8e4
DRM = mybir.MatmulPerfMode.DoubleRow
SVT = 32.0     # vt8 fp8 scale
SSM = 64.0     # sm2t fp8 scale  (attn@v PSUM scale = SVT*SSM = 2048)

B, C, RES = 64, 384, 14
NH, KD, DV = 8, 32, 128
N = RES * RES            # 196
SCALE = KD ** -0.5
NCORES = 8
BL = B // NCORES         # 8 items per core
NS = 14                  # query-window size
NW = N // NS             # 14 windows
P112 = NH * NS           # 112 partitions for attn tiles
WGRPS = (4, 4, 3, 3)     # th2 psum window grouping (sum = 14)
IDENT = mybir.ActivationFunctionType.Identity
COPY = mybir.ActivationFunctionType.Copy
EXP = mybir.ActivationFunctionType.Exp
RELU = mybir.ActivationFunctionType.Relu
MULT = mybir.AluOpType.mult
ADD = mybir.AluOpType.add

_CACHE = {}


def _build_nc():
    nc = bacc.Bacc(None, target_bir_lowering=False)

    x8 = nc.declare_dram_parameter("x8", [BL, C, N], F32, isOutput=False)
    qkw_t = nc.declare_dram_parameter("qkw_t", [128, 3, 512], F32, isOutput=False)
    qkb_p = nc.declare_dram_parameter("qkb_p", [128, 4], F32, isOutput=False)
    vw_t = nc.declare_dram_parameter("vw_t", [128, 3, 1024], F32, isOutput=False)
    vb_p = nc.declare_dram_parameter("vb_p", [128, 8], F32, isOutput=False)
    vlw_p = nc.declare_dram_parameter("vlw_p", [128, 8, 9], F32, isOutput=False)
    obias_c = nc.declare_dram_parameter("obias_c", [128, 8], F32, isOutput=False)
    th2b_p = nc.declare_dram_parameter("th2b_p", [128, 8], F32, isOutput=False)
    th1s_p = nc.declare_dram_parameter("th1s_p", [128, 2, 8], F32, isOutput=False)
    t2k_d = nc.declare_dram_parameter("t2k", [P112, P112], BF, isOutput=False)
    bias1_d = nc.declare_dram_parameter("bias1_il", [P112, NW, N], BF, isOutput=False)
    i112_d = nc.declare_dram_parameter("i112", [P112, P112], BF, isOutput=False)
    i128_d = nc.declare_dram_parameter("i128", [128, 128], BF, isOutput=False)
    projw_t = nc.declare_dram_parameter("projw_t", [128, 8, 384], BF, isOutput=False)
    dwdiag_d = nc.declare_dram_parameter("dwdiag", [128, 8, 6, 128], BF, isOutput=False)
    projb_p = nc.declare_dram_parameter("projb_p", [128, 3], F32, isOutput=False)

    y8 = nc.declare_dram_parameter("y8", [BL, C, N], F32, isOutput=True)

    with tile.TileContext(nc) as tc:
        with (
            tc.tile_pool(name="const", bufs=1) as const,
            tc.tile_pool(name="pairp", bufs=2) as pairp,
            tc.tile_pool(name="itemp", bufs=3) as itemp,
            tc.tile_pool(name="egrp", bufs=6) as epool,
            tc.tile_pool(name="t2rp", bufs=6) as t2rpool,
            tc.tile_pool(name="ps392", bufs=2, space="PSUM") as ps392,
            tc.tile_pool(name="psA", bufs=2, space="PSUM") as psA,
            tc.tile_pool(name="psUT", bufs=1, space="PSUM") as psUT,
            tc.tile_pool(name="psVT", bufs=1, space="PSUM") as psVT,
            tc.tile_pool(name="psVTb", bufs=1, space="PSUM") as psVTb,
        ):
            # ---------------- early input prefetch (pair 0) ----------------
            x2_first = pairp.tile([128, 3, 2, N], F32R, tag="x2")
            for c in range(3):
                for i2 in range(2):
                    nc.sync.dma_start(
                        out=x2_first[:, c, i2],
                        in_=x8[i2, c * 128 : (c + 1) * 128].bitcast(F32R),
                    )
            # ---------------- constants ----------------
            qkw_sb = const.tile([128, 3, 512], F32R)
            for c in range(3):
                nc.sync.dma_start(
                    out=qkw_sb[:, c], in_=qkw_t[:, c].bitcast(F32R)
                )
            vw_sb = const.tile([128, 3, 1024], F32R)
            for c in range(3):
                nc.sync.dma_start(out=vw_sb[:, c], in_=vw_t[:, c].bitcast(F32R))
            qkb_sb = const.tile([128, 4], F32)
            nc.sync.dma_start(out=qkb_sb[:], in_=qkb_p[:])
            vb_sb = const.tile([128, 8], F32)
            nc.sync.dma_start(out=vb_sb[:], in_=vb_p[:])
            vlw_sb = const.tile([128, 8, 9], F32)
            nc.sync.dma_start(out=vlw_sb[:], in_=vlw_p[:])
            obias_sb = const.tile([128, 8], F32)
            nc.sync.dma_start(out=obias_sb[:], in_=obias_c[:])
            th2b_sb = const.tile([128, 8], F32)
            nc.sync.dma_start(out=th2b_sb[:], in_=th2b_p[:])
            th1s_sb = const.tile([128, 2, 8], F32)
            nc.sync.dma_start(out=th1s_sb[:], in_=th1s_p[:])
            projb_sb = const.tile([128, 3], F32)
            nc.sync.dma_start(out=projb_sb[:], in_=projb_p[:])

            t2k_sb = const.tile([P112, P112], BF)
            nc.sync.dma_start(out=t2k_sb[:], in_=t2k_d[:])
            i112_sb = const.tile([P112, P112], BF)
            nc.sync.dma_start(out=i112_sb[:], in_=i112_d[:])
            i128_sb = const.tile([128, 128], BF)
            nc.sync.dma_start(out=i128_sb[:], in_=i128_d[:])
            projw_sb = const.tile([128, 8, 384], BF)
            nc.sync.dma_start(out=projw_sb[:], in_=projw_t[:])
            dwdiag_sb = const.tile([128, 8, 6, 128], BF)
            nc.sync.dma_start(out=dwdiag_sb[:], in_=dwdiag_d[:])
            bias1_sb = const.tile([P112, NW, N], BF)
            nc.sync.dma_start(out=bias1_sb[:], in_=bias1_d[:])

            # ---------------- per item-pair ----------------
            for pr in range(BL // 2):
                if pr == 0:
                    x2 = x2_first
                else:
                    x2 = pairp.tile([128, 3, 2, N], F32R, tag="x2")
                    for i2 in range(2):
                        nc.sync.dma_start(
                            out=x2[:, :, i2],
                            in_=x8[2 * pr + i2]
                            .rearrange("(c p) n -> p c n", p=128)
                            .bitcast(F32R),
                        )

                # --- Q,K projections: chunks mt 0,1 = q; 2,3 = k ---
                qk = pairp.tile([128, 4, 2, N], BF, tag="qk")
                for mt in range(4):
                    pp = ps392.tile([128, 392], F32, tag="mm392")
                    for c in range(3):
                        nc.tensor.matmul(
                            pp[:],
                            qkw_sb[:, c, mt * 128 : (mt + 1) * 128],
                            x2[:, c].rearrange("p i n -> p (i n)"),
                            start=(c == 0),
                            stop=(c == 2),
                        )
                    if mt < 2:
                        nc.vector.tensor_scalar(
                            qk[:, mt].rearrange("p i n -> p (i n)"),
                            pp[:],
                            qkb_sb[:, mt : mt + 1],
                            None,
                            ADD,
                        )
                    else:
                        nc.scalar.activation(
                            qk[:, mt].rearrange("p i n -> p (i n)"),
                            pp[:],
                            IDENT,
                            bias=qkb_sb[:, mt : mt + 1],
                            scale=1.0,
                        )

                # --- V projection, natural layout; +vb and rowsum(v) at evict ---
                avb = pairp.tile([128, 8, 2, N], BF, tag="avb")
                vsum = pairp.tile([128, 8, 2], F32, tag="vsum")
                for ch in range(8):
                    pp = ps392.tile([128, 392], F32, tag="mm392")
                    for c in range(3):
                        nc.tensor.matmul(
                            pp[:],
                            vw_sb[:, c, ch * 128 : (ch + 1) * 128],
                            x2[:, c].rearrange("p i n -> p (i n)"),
                            start=(c == 0),
                            stop=(c == 2),
                        )
                    for i2 in range(2):
                        nc.scalar.activation(
                            avb[:, ch, i2],
                            pp[:, i2 * N : (i2 + 1) * N],
                            IDENT,
                            bias=vb_sb[:, ch : ch + 1],
                            scale=1.0,
                            accum_out=vsum[:, ch, i2 : i2 + 1],
                        )

                # obias[:, ch, i2] = rowsum(v)*th2_b[h] + (vb*s2[h] + vlb[h])
                obias = pairp.tile([128, 8, 2], F32, tag="obias")
                for ch in range(8):
                    for i2 in range(2):
                        nc.vector.tensor_scalar(
                            obias[:, ch, i2 : i2 + 1],
                            vsum[:, ch, i2 : i2 + 1],
                            th2b_sb[:, ch : ch + 1],
                            obias_sb[:, ch : ch + 1],
                            MULT,
                            ADD,
                        )

                # --- depthwise 3x3 conv: 9 per-partition-scalar taps (DVE) ---
                vloc = pairp.tile([128, 8, 2, N], BF, tag="vloc")
                for ch in range(8):
                    # center tap: both items in one op
                    nc.vector.tensor_scalar(
                        vloc[:, ch].rearrange("p i n -> p (i n)"),
                        avb[:, ch].rearrange("p i n -> p (i n)"),
                        vlw_sb[:, ch, 4:5],
                        None,
                        MULT,
                    )
                    sr = avb[:, ch].rearrange("p i (y x) -> p (i y) x", x=RES)
                    dr = vloc[:, ch].rearrange("p i (y x) -> p (i y) x", x=RES)
                    for dx in (-1, 1):  # dy == 0: (i y) merged, both items
                        t = 3 + (dx + 1)
                        x0, x1 = max(0, -dx), min(RES, RES - dx)
                        d = dr[:, :, x0:x1]
                        s = sr[:, :, x0 + dx : x1 + dx]
                        nc.vector.scalar_tensor_tensor(
                            d, s, vlw_sb[:, ch, t : t + 1], d, MULT, ADD
                        )


                orelu = pairp.tile([128, 8, 2, N], BF, tag="orelu")
                sm2ts = []
                vts = []

                for i2 in range(2):
                    # --- V^T direct (98/98 m-tiles, fp8*SVT for DoubleRow) ---
                    vt8 = itemp.tile([98, 2, 1024], F8, tag="vta")
                    for half in range(2):
                        vp_a = psVT.tile([98, 512], F32, tag="vtps_a")
                        vp_b = psVTb.tile([98, 512], F32, tag="vtps_b")
                        for c in range(3):
                            nc.tensor.matmul(
                                vp_a[:],
                                x2[:, c, i2, 0:98],
                                vw_sb[:, c, half * 512 : (half + 1) * 512],
                                start=(c == 0),
                                stop=(c == 2),
                            )
                            nc.tensor.matmul(
                                vp_b[:],
                                x2[:, c, i2, 98:N],
                                vw_sb[:, c, half * 512 : (half + 1) * 512],
                                start=(c == 0),
                                stop=(c == 2),
                            )
                        nc.vector.tensor_scalar(
                            vt8[:, 0, half * 512 : (half + 1) * 512],
                            vp_a[:], SVT, None, MULT,
                        )
                        nc.vector.tensor_scalar(
                            vt8[:, 1, half * 512 : (half + 1) * 512],
                            vp_b[:], SVT, None, MULT,
                        )

                    # --- th1-scaled Q copies, window-major: [c2, w, (h,ns)] ---
                    qp = itemp.tile([128, 2, NW, NH, NS], BF, tag="qp")
                    for c2 in range(2):
                        for g in range(NH):
                            nc.vector.tensor_scalar(
                                qp[:, c2, :, g, :],
                                qk[:, c2, i2].rearrange("p (w ns) -> p w ns", ns=NS),
                                th1s_sb[:, c2, g : g + 1],
                                None,
                                MULT,
                            )

                    # --- per window-group: scores+exp, recip+T2R, th2+evict ---
                    dsum = itemp.tile([P112, NW], F32, tag="dsum")
                    rbuf = itemp.tile([P112, NW], F32, tag="rbuf")
                    rbf = itemp.tile([P112, NW], BF, tag="rbf")
                    sm2t8 = itemp.tile([98, 2, NW, NH, NS], F8, tag="sm2a")
                    w0 = 0
                    for gsz in WGRPS:
                        egrp = epool.tile([P112, 4, N], BF, tag="egrp")
                        subs = [(0, min(2, gsz))] + ([(2, gsz)] if gsz > 2 else [])
                        for s0, s1 in subs:
                            sps = []
                            for wi in range(s0, s1):
                                sp = psA.tile([P112, N], F32, tag="attnps")
                                nc.tensor.matmul(
                                    sp[:], i112_sb[:], bias1_sb[:, w0 + wi],
                                    start=True, stop=False,
                                )
                                sps.append(sp)
                            for wi in range(s0, s1):
                                sp = sps[wi - s0]
                                for c2 in range(2):
                                    nc.tensor.matmul(
                                        sp[:],
                                        qp[:, c2, w0 + wi],
                                        qk[:, 2 + c2, i2],
                                        start=False,
                                        stop=(c2 == 1),
                                    )
                                nc.scalar.activation(
                                    egrp[:, wi], sp[:], EXP,
                                    accum_out=dsum[:, w0 + wi : w0 + wi + 1],
                                )
                        nc.vector.reciprocal(
                            rbuf[:, w0 : w0 + gsz], dsum[:, w0 : w0 + gsz]
                        )
                        nc.vector.tensor_copy(
                            rbf[:, w0 : w0 + gsz], rbuf[:, w0 : w0 + gsz]
                        )
                        t2r = t2rpool.tile([P112, 4, P112], BF, tag="t2r")
                        nc.vector.tensor_tensor(
                            t2r[:, 0:gsz],
                            t2k_sb[:]
                            .rearrange("p (o j) -> p o j", o=1)
                            .broadcast_to([P112, gsz, P112]),
                            rbf[:, w0 : w0 + gsz]
                            .rearrange("p (w o) -> p w o", o=1)
                            .broadcast_to([P112, gsz, P112]),
                            MULT,
                        )
                        uta = psUT.tile([98, 4, P112], F32, tag="utps_a")
                        utb = psUT.tile([98, 4, P112], F32, tag="utps_b")
                        for wi in range(gsz):
                            nc.tensor.matmul(
                                uta[:, wi],
                                egrp[:, wi, 0:98],
                                t2r[:, wi],
                                start=True,
                                stop=True,
                            )
                            nc.tensor.matmul(
                                utb[:, wi],
                                egrp[:, wi, 98:N],
                                t2r[:, wi],
                                start=True,
                                stop=True,
                            )
                        dsta = sm2t8[:, 0, w0 : w0 + gsz].rearrange(
                            "m w h ns -> m w (h ns)"
                        )
                        dstb = sm2t8[:, 1, w0 : w0 + gsz].rearrange(
                            "m w h ns -> m w (h ns)"
                        )
                        if w0 % 2 == 0:
                            nc.vector.tensor_scalar(dsta, uta[:, 0:gsz], SSM, None, MULT)
                            nc.vector.tensor_scalar(dstb, utb[:, 0:gsz], SSM, None, MULT)
                        else:
                            nc.scalar.activation(dsta, uta[:, 0:gsz], IDENT, bias=0.0, scale=SSM)
                            nc.vector.tensor_scalar(dstb, utb[:, 0:gsz], SSM, None, MULT)
                        w0 += gsz

                    sm2ts.append(sm2t8)
                    vts.append(vt8)

                # --- attn@v + vlocal preload + PE dwconv taps; ReLU+obias ---
                DIAG_TAPS = []  # (tap slot in dwdiag, dy, dx)
                for slot, (dy, dx) in enumerate(
                    [(-1, -1), (-1, 0), (-1, 1), (1, -1), (1, 0), (1, 1)]
                ):
                    DIAG_TAPS.append((slot, dy, dx))
                for h in range(NH):
                    op = ps392.tile([128, 2, N], F32, tag="mm392")
                    nc.tensor.matmul(
                        op[:].rearrange("p i n -> p (i n)"),
                        i128_sb[:],
                        vloc[:, h].rearrange("p i n -> p (i n)"),
                        start=True,
                        stop=False,
                    )
                    # dwconv dy=+-1 taps, both items per matmul
                    for slot, dy, dx in DIAG_TAPS:
                        y0, y1 = max(0, -dy), min(RES, RES - dy)
                        x0, x1 = max(0, -dx), min(RES, RES - dx)
                        if dx == 0:
                            d = op[:].rearrange("p i (y x) -> p i (y x)", x=RES)[
                                :, :, y0 * RES : y1 * RES
                            ]
                            s = avb[:, h][:, :, (y0 + dy) * RES : (y1 + dy) * RES]
                            nc.tensor.matmul(
                                d, dwdiag_sb[:, h, slot], s, start=False, stop=False
                            )
                        else:
                            for i2 in range(2):
                                d = op[:, i2].rearrange("p (y x) -> p y x", x=RES)[
                                    :, y0:y1, x0:x1
                                ]
                                s = avb[:, h, i2].rearrange(
                                    "p (y x) -> p y x", x=RES
                                )[:, y0 + dy : y1 + dy, x0 + dx : x1 + dx]
                                nc.tensor.matmul(
                                    d,
                                    dwdiag_sb[:, h, slot],
                                    s,
                                    start=False,
                                    stop=False,
                                )
                    for i2 in range(2):
                        nc.tensor.matmul(
                            op[:, i2].rearrange("p (w ns) -> p w ns", ns=NS),
                            vts[i2][:, :, h * 128 : (h + 1) * 128],
                            sm2ts[i2][:, :, :, h],
                            start=False,
                            stop=(i2 == 1),
                            perf_mode=DRM,
                        )
                    nc.scalar.activation(
                        orelu[:, h, 0],
                        op[:, 0],
                        RELU,
                        bias=obias[:, h, 0:1],
                        scale=1.0 / (SVT * SSM),
                    )
                    nc.scalar.activation(
                        orelu[:, h, 1],
                        op[:, 1],
                        RELU,
                        bias=obias[:, h, 1:2],
                        scale=1.0 / (SVT * SSM),
                    )

                # --- final projection (pair-wide, bf16) ---
                out_sb = pairp.tile([128, 3, 2, N], F32, tag="out")
                for mt in range(3):
                    pp = ps392.tile([128, 392], F32, tag="mm392")
                    for ch in range(8):
                        nc.tensor.matmul(
                            pp[:],
                            projw_sb[:, ch, mt * 128 : (mt + 1) * 128],
                            orelu[:, ch].rearrange("p i n -> p (i n)"),
                            start=(ch == 0),
                            stop=(ch == 7),
                        )
                    nc.scalar.activation(
                        out_sb[:, mt].rearrange("p i n -> p (i n)"),
                        pp[:],
                        IDENT,
                        bias=projb_sb[:, mt : mt + 1],
                        scale=1.0,
                    )
                for i2 in range(2):
                    for mt in range(3):
                        nc.sync.dma_start(
                            out=y8[2 * pr + i2, mt * 128 : (mt + 1) * 128],
                            in_=out_sb[:, mt, i2],
                        )

    nc.compile()
    return nc


def _host_prep(qw, qb, kw, kb, vw, vb, vlw, vlb, th1_w, th1_b, th2_w, th2_b,
               projw, projb, bias_seg, bias_idxs):
    f = np.float32
    qw, qb, kw, kb = (np.asarray(a, f) for a in (qw, qb, kw, kb))
    vw, vb, vlw, vlb = (np.asarray(a, f) for a in (vw, vb, vlw, vlb))
    th1_w, th1_b, th2_w, th2_b = (
        np.asarray(a, f) for a in (th1_w, th1_b, th2_w, th2_b)
    )
    projw, projb = np.asarray(projw, f), np.asarray(projb, f)
    bias_seg = np.asarray(bias_seg, f)
    bias_idxs = np.asarray(bias_idxs)

    qkw = np.concatenate([qw * SCALE, kw], axis=0)                     # [512,384]
    qkw_t = np.ascontiguousarray(qkw.T.reshape(3, 128, 512).transpose(1, 0, 2))
    qkb = np.concatenate([qb * SCALE, kb])
    qkb_p = np.ascontiguousarray(qkb.reshape(4, 128).T)

    vw_t = np.ascontiguousarray(vw.T.reshape(3, 128, 1024).transpose(1, 0, 2))
    vb_p = np.ascontiguousarray(vb.reshape(8, 128).T)

    vlw9 = vlw.reshape(1024, 9)
    vlw_p = np.ascontiguousarray(
        vlw9.reshape(8, 128, 9).transpose(1, 0, 2)
    ) * 2048.0

    # obias constant part: vb*s2[h] + vlb ; s2[h] = sum_g th2[h,g]
    s2 = th2_w.sum(axis=1)                                             # [8]
    obias_full = vb * np.repeat(s2, DV) + vlb                          # [1024]
    obias_c = np.ascontiguousarray(obias_full.reshape(8, 128).T)
    th2b_p = np.ascontiguousarray(
        np.repeat(th2_b[:, None], 128, axis=1).T
    )                                                                  # [128,8]

    th1s = np.repeat(th1_w.T, KD, axis=0)                              # [256,8]
    th1s_p = np.ascontiguousarray(th1s.reshape(2, 128, 8).transpose(1, 0, 2))

    t2k = np.kron(th2_w.T, np.eye(NS, dtype=f))                        # [112,112]

    bias_full = bias_seg[:, bias_idxs]                                 # [8,196,196]
    bias1 = np.einsum("hg,gnm->hnm", th1_w, bias_full)
    bias1 += th1_b[:, None, None]
    bias1_il = np.ascontiguousarray(
        bias1.reshape(NH, NW, NS, N).transpose(0, 2, 1, 3).reshape(P112, NW, N)
    )

    projw_t = np.ascontiguousarray(
        projw.T.reshape(8, 128, 384).transpose(1, 0, 2)
    )

    # diag-stationary weights for the 6 dy=+-1 dwconv taps on TensorE
    taps6 = [0, 1, 2, 6, 7, 8]  # (dy,dx): (-1,-1),(-1,0),(-1,1),(1,-1),(1,0),(1,1)
    dwdiag = np.zeros((128, 8, 6, 128), f)
    eye = np.eye(128, dtype=f)
    for ch in range(8):
        for si, t in enumerate(taps6):
            dwdiag[:, ch, si, :] = (
                eye * vlw9[ch * 128 : (ch + 1) * 128, t][:, None] * 2048.0
            )
    projb_p = np.ascontiguousarray(projb.reshape(3, 128).T)

    nbf = ml_dtypes.bfloat16
    return dict(
        qkw_t=qkw_t, qkb_p=qkb_p, vw_t=vw_t, vb_p=vb_p, vlw_p=vlw_p,
        obias_c=obias_c, th2b_p=th2b_p, th1s_p=th1s_p,
        t2k=t2k.astype(nbf), bias1_il=bias1_il.astype(nbf),
        dwdiag=dwdiag.astype(nbf), i112=np.eye(P112, dtype=nbf),
        i128=np.eye(128, dtype=nbf), projw_t=projw_t.astype(nbf),
        projb_p=projb_p,
    )


def kernel(**inputs):
    x = np.asarray(inputs["x"], np.float32)
    consts = _host_prep(
        inputs["qw"], inputs["qb"], inputs["kw"], inputs["kb"],
        inputs["vw"], inputs["vb"], inputs["vlw"], inputs["vlb"],
        inputs["th1_w"], inputs["th1_b"], inputs["th2_w"], inputs["th2_b"],
        inputs["projw"], inputs["projb"], inputs["bias_seg"], inputs["bias_idxs"],
    )
    if "nc" not in _CACHE:
        _CACHE["nc"] = _build_nc()
    nc = _CACHE["nc"]

    xs = np.ascontiguousarray(x.reshape(NCORES, BL, C, N))
    in_maps = [dict(consts, x8=xs[i]) for i in range(NCORES)]
    res = run_bass_kernel_spmd(
        nc, in_maps, list(range(NCORES)), **_CACHE.get("run_kwargs", {})
    )
    _CACHE["last_results"] = res
    out = np.stack([np.asarray(res.results[i]["y8"]) for i in range(NCORES)])
    return out.reshape(B, C, RES, RES).astype(np.float32)

